# revision 10
# baseline (speedup 1.0000x reference)
"""CBOW forward (embedding lookup -> ReLU -> vocab projection) on 8 TRN2 cores.

Full inputs in, full output out.  Sharding: pure data-parallel over the
batch.  Core c owns rows [c*256, (c+1)*256): it gathers + reduces the
context embeddings for its two 128-row blocks (16 HW-DGE indirect-DMA
calls -- the indirect path costs ~1.4us of gpsimd issue time per call
regardless of size, so call count is what matters), relu's the
transposed result into four resident bf16 hT tiles, then computes
out[own, :] = h @ W2.T for the FULL vocab, streaming W2T through SBUF
in [128, 1024] bf16 tiles.

Why not vocab-shard layer 2 (8x less W2 traffic)?  That needs an
AllGather of h, and a measured probe puts the fixed cost of any
collective in this runtime at ~95us (rendezvous + init) -- more than
the W2 streaming it saves.  With no cross-core dependency, per-core
launch skew doesn't stack either.

The harness accuracy gate is rel_err < 2e-2; the whole pipeline runs in
bf16 (measured ~5e-3): W1T is pre-scaled by 1/8 (exact in bf16), the
context sum is a 3-level bf16 DVE tree, layer 2 is a single bf16 term
with fp32 PSUM accumulate, and the output is written bf16 (halves the
dominant HBM traffic; host upcasts).  PSUM eviction (fp32->bf16,
~1.1ns/col/partition) alternates DVE/Scalar per tile so neither engine
paces the loop.  Output accumulates in [128, 8192] chunks so every DMA
descriptor row is 16KB contiguous.

Duplicate context indices use scatter-SET semantics (count once): the
host redirects duplicate occurrences to an appended all-zero row of
W1T.  b1/b2 are zero in this problem (spec fill=zeros); a general
fallback path (scalar-relu with b1 bias, streamed b2-add evictions)
compiles only if nonzero biases ever show up.
"""

from contextlib import ExitStack

import numpy as np
import ml_dtypes

import concourse.bacc as bacc
import concourse.bass as bass
import concourse.mybir as mybir
import concourse.tile as tile
from concourse.masks import make_identity

# Problem shape (hardcoded per the task contract).
N = 2048          # batch
J = 8             # context window (2*CTX)
D = 256           # hidden
V = 50000         # vocab
C = 8             # cores

P = 128
LB = N // (C * P)  # local 128-row blocks per core = 2
VT = 1024          # matmul/eviction tile width (two PSUM banks)
CHW = 4096         # output chunk width (8KB bf16 rows -> fat DMA descriptors)

F32 = mybir.dt.float32
BF16 = mybir.dt.bfloat16
I32 = mybir.dt.int32

_CACHE = {}


def _build(zero_bias=True):
    """Build + compile the single-core SPMD Bass program."""
    key = ("nc", zero_bias)
    if key in _CACHE:
        return _CACHE[key]

    nc = bacc.Bacc("TRN2", target_bir_lowering=False, debug=False, num_devices=C)

    idx_d = nc.dram_tensor("idx", [P, LB * J], I32, kind="ExternalInput")
    w1t_d = nc.dram_tensor("w1t", [V + 1, D], BF16, kind="ExternalInput")
    w2t_d = nc.dram_tensor("w2t", [D, V], BF16, kind="ExternalInput")
    out_d = nc.dram_tensor("out", [LB * P, V], BF16, kind="ExternalOutput")
    if not zero_bias:
        b1_d = nc.dram_tensor("b1", [2, P, 1], F32, kind="ExternalInput")
        b2_d = nc.dram_tensor("b2", [1, V], F32, kind="ExternalInput")

    # output chunks of 8192 cols; vtiles of 1024 within a chunk (tail 848)
    chunks = [(k, min(CHW, V - k)) for k in range(0, V, CHW)]

    with tile.TileContext(nc) as tc, ExitStack() as ctx:
        const = ctx.enter_context(tc.tile_pool(name="const", bufs=1))
        w2pool = ctx.enter_context(tc.tile_pool(name="w2", bufs=48))
        gpool = ctx.enter_context(tc.tile_pool(name="g8", bufs=2))
        t4pool = ctx.enter_context(tc.tile_pool(name="t4", bufs=2))
        t2pool = ctx.enter_context(tc.tile_pool(name="t2", bufs=2))
        hpool = ctx.enter_context(tc.tile_pool(name="hraw", bufs=2))
        opool = ctx.enter_context(tc.tile_pool(name="out", bufs=4))
        b2pool = ctx.enter_context(tc.tile_pool(name="b2s", bufs=6))
        ps_s = ctx.enter_context(tc.tile_pool(name="ps_s", bufs=2, space="PSUM"))
        ps_b = ctx.enter_context(tc.tile_pool(name="ps_b", bufs=3, space="PSUM"))

        # ---- resident tensors -------------------------------------------
        idx_sb = const.tile([P, LB * J], I32, tag="idx")
        nc.sync.dma_start(idx_sb[:], idx_d[:])
        ident = const.tile([P, P], BF16, tag="ident")
        make_identity(nc, ident[:])
        if not zero_bias:
            b1t = [const.tile([P, 1], F32, tag=f"b1{h}", name=f"b1{h}")
                   for h in (0, 1)]
            for h in (0, 1):
                nc.sync.dma_start(b1t[h][:], b1_d[h])

        # ---- layer 1: own two blocks ------------------------------------
        def layer1(lb):
            # pairwise tree: each add only waits on two gathers, so the
            # reduction races the (serial, ~1.1us each) gather issue stream
            g8 = gpool.tile([P, J, D], BF16, tag=f"g8{lb}", name="g8")
            for j in range(J):
                nc.gpsimd.indirect_dma_start(
                    out=g8[:, j, :],
                    out_offset=None,
                    in_=w1t_d[:],
                    in_offset=bass.IndirectOffsetOnAxis(
                        ap=idx_sb[:, lb * J + j:lb * J + j + 1], axis=0),
                )
            t4 = t4pool.tile([P, 4, D], BF16, tag=f"t4{lb}", name="t4")
            for q in range(4):
                nc.vector.tensor_add(t4[:, q, :], g8[:, 2 * q, :],
                                     g8[:, 2 * q + 1, :])
            t2 = t2pool.tile([P, 2, D], BF16, tag=f"t2{lb}", name="t2")
            for q in range(2):
                nc.vector.tensor_add(t2[:, q, :], t4[:, 2 * q, :],
                                     t4[:, 2 * q + 1, :])
            h_raw = hpool.tile([P, D], BF16, tag=f"hraw{lb}", name="h_raw")
            nc.vector.tensor_add(h_raw[:], t2[:, 0, :], t2[:, 1, :])

            hts = []
            for h in (0, 1):
                pt = ps_s.tile([P, P], BF16, tag="ps", name="pt")
                nc.tensor.transpose(pt[:], h_raw[:, h * P:(h + 1) * P],
                                    ident[:])
                ht = const.tile([P, P], BF16, tag=f"ht{lb}{h}",
                                name=f"ht{lb}{h}")
                if zero_bias:
                    nc.vector.tensor_scalar_max(ht[:], pt[:], 0.0)
                else:
                    nc.scalar.activation(ht[:], pt[:],
                                         mybir.ActivationFunctionType.Relu,
                                         bias=b1t[h][:], scale=1.0)
                hts.append(ht)
            return hts

        ht = [layer1(lb) for lb in range(LB)]

        # ---- layer 2: stream W2T, both blocks per vtile ------------------
        def fetch_w2(v0, vw):
            pair = []
            for h in (0, 1):
                w2 = w2pool.tile([P, VT], BF16, tag="w2", name="w2")
                nc.sync.dma_start(w2[:, :vw], w2t_d[h * P:(h + 1) * P,
                                                    v0:v0 + vw])
                pair.append(w2)
            if not zero_bias:
                b2s = b2pool.tile([P, VT], F32, tag="b2s", name="b2s")
                nc.sync.dma_start(b2s[:, :vw],
                                  b2_d[:, v0:v0 + vw].to_broadcast([P, vw]))
                pair.append(b2s)
            return pair

        vtiles = []
        for k0, kw in chunks:
            for v0 in range(k0, k0 + kw, VT):
                vtiles.append((v0, min(VT, k0 + kw - v0)))

        PREF = 23
        w2f = {i: fetch_w2(*vtiles[i]) for i in range(PREF)}
        ob = {}
        vt_i = 0
        for k, (k0, kw) in enumerate(chunks):
            for lb in range(LB):
                ob[lb] = opool.tile([P, CHW], BF16, tag=f"ob{lb}",
                                    name=f"ob{lb}")
            nvt = (kw + VT - 1) // VT
            for t in range(nvt):
                v0, vw = vtiles[vt_i]
                if vt_i + PREF < len(vtiles):
                    w2f[vt_i + PREF] = fetch_w2(*vtiles[vt_i + PREF])
                w2pair = w2f.pop(vt_i)
                c0 = v0 - k0
                for lb in range(LB):
                    po = ps_b.tile([P, VT], F32, tag="po", name="po")
                    for sub in range(0, vw, 512):
                        sw = min(512, vw - sub)
                        for h in (0, 1):
                            nc.tensor.matmul(
                                po[:, sub:sub + sw],
                                lhsT=ht[lb][h][:],
                                rhs=w2pair[h][:, sub:sub + sw],
                                start=(h == 0),
                                stop=(h == 1))
                    if not zero_bias:
                        nc.vector.tensor_add(ob[lb][:, c0:c0 + vw],
                                             po[:, :vw], w2pair[2][:, :vw])
                    elif (lb + t) % 2 == 0:
                        nc.vector.tensor_scalar_add(ob[lb][:, c0:c0 + vw],
                                                    po[:, :vw], 0.0)
                    else:
                        nc.scalar.copy(ob[lb][:, c0:c0 + vw], po[:, :vw])
                vt_i += 1
            nc.sync.dma_start(out_d[0:P, k0:k0 + kw], ob[0][:, :kw])
            nc.scalar.dma_start(out_d[P:2 * P, k0:k0 + kw], ob[1][:, :kw])

    nc.compile()
    _CACHE[key] = nc
    return nc


def _host_prep(inputs, W1, b1, W2, b2, zero_bias):
    x = np.asarray(inputs)
    assert x.shape == (N, J) and x.dtype == np.int32

    # duplicate mask: scatter-SET semantics -> only first occurrence counts;
    # duplicates are redirected to the all-zero row V of the augmented W1T.
    dup = np.zeros((N, J), dtype=bool)
    for j in range(1, J):
        dup[:, j] = (x[:, :j] == x[:, j:j + 1]).any(axis=1)
    xd = np.where(dup, V, x).astype(np.int32)

    w1 = np.asarray(W1, dtype=np.float32)
    w1t = np.concatenate([w1.T / J, np.zeros((1, D), np.float32)], axis=0)
    w1t = np.ascontiguousarray(w1t).astype(ml_dtypes.bfloat16)   # [V+1, D]

    w2t = np.ascontiguousarray(
        np.asarray(W2, dtype=np.float32).T).astype(ml_dtypes.bfloat16)

    in_maps = []
    for c in range(C):
        # idx2[p, lb*J + j] = xd[c*256 + lb*128 + p, j]
        xc = xd[c * LB * P:(c + 1) * LB * P]
        idx2 = np.ascontiguousarray(
            xc.reshape(LB, P, J).transpose(1, 0, 2).reshape(P, LB * J))
        m = {
            "idx": idx2,
            "w1t": w1t,
            "w2t": w2t,
        }
        if not zero_bias:
            m["b1"] = np.ascontiguousarray(
                np.asarray(b1, dtype=np.float32).reshape(2, P, 1))
            m["b2"] = np.ascontiguousarray(
                np.asarray(b2, dtype=np.float32).reshape(1, V))
        in_maps.append(m)
    return in_maps


def run(inputs, W1, b1, W2, b2, trace=False):
    from concourse.bass_utils import run_bass_kernel_spmd

    zero_bias = not (np.any(np.asarray(b1)) or np.any(np.asarray(b2)))
    nc = _build(zero_bias)
    in_maps = _host_prep(inputs, W1, b1, W2, b2, zero_bias)
    res = run_bass_kernel_spmd(nc, in_maps, core_ids=list(range(C)), trace=trace)
    out = np.concatenate([res.results[c]["out"] for c in range(C)], axis=0)
    return out.astype(np.float32), res


def kernel(inputs, W1, b1, W2, b2):
    out, _ = run(inputs, W1, b1, W2, b2, trace=False)
    return out


# revision 11
# speedup vs baseline: 1.1477x; 1.1477x over previous
"""CBOW forward (embedding lookup -> ReLU -> vocab projection) on 8 TRN2 cores.

Full inputs in, full output out.  Sharding: pure data-parallel over the
batch.  Core c owns rows [c*256, (c+1)*256): it gathers + reduces the
context embeddings for its two 128-row blocks (16 HW-DGE indirect-DMA
calls -- the indirect path costs ~1.4us of gpsimd issue time per call
regardless of size, so call count is what matters), relu's the
transposed result into four resident bf16 hT tiles, then computes
out[own, :] = h @ W2.T for the FULL vocab, streaming W2T through SBUF
in [128, 1024] bf16 tiles.

Why not vocab-shard layer 2 (8x less W2 traffic)?  That needs an
AllGather of h, and a measured probe puts the fixed cost of any
collective in this runtime at ~95us (rendezvous + init) -- more than
the W2 streaming it saves.  With no cross-core dependency, per-core
launch skew doesn't stack either.

The harness accuracy gate is rel_err < 2e-2; the whole pipeline runs in
bf16 (measured ~5e-3): W1T is pre-scaled by 1/8 (exact in bf16), the
context sum is a 3-level bf16 DVE tree, layer 2 is a single bf16 term
with fp32 PSUM accumulate, and the output is written bf16 (halves the
dominant HBM traffic; host upcasts).  PSUM eviction (fp32->bf16,
~1.1ns/col/partition) alternates DVE/Scalar per tile so neither engine
paces the loop.  Output accumulates in [128, 8192] chunks so every DMA
descriptor row is 16KB contiguous.

Duplicate context indices use scatter-SET semantics (count once): the
host redirects duplicate occurrences to an appended all-zero row of
W1T.  b1/b2 are zero in this problem (spec fill=zeros); a general
fallback path (scalar-relu with b1 bias, streamed b2-add evictions)
compiles only if nonzero biases ever show up.
"""

from contextlib import ExitStack

import numpy as np
import ml_dtypes

import concourse.bacc as bacc
import concourse.bass as bass
import concourse.mybir as mybir
import concourse.tile as tile
from concourse.masks import make_identity

# Problem shape (hardcoded per the task contract).
N = 2048          # batch
J = 8             # context window (2*CTX)
D = 256           # hidden
V = 50000         # vocab
C = 8             # cores

P = 128
LB = N // (C * P)  # local 128-row blocks per core = 2
VT = 1024          # matmul/eviction tile width (two PSUM banks)
CHW = 8192         # output chunk width (16KB bf16 rows -> fat DMA descriptors)

F32 = mybir.dt.float32
BF16 = mybir.dt.bfloat16
I32 = mybir.dt.int32

_CACHE = {}


def _build(zero_bias=True):
    """Build + compile the single-core SPMD Bass program."""
    key = ("nc", zero_bias)
    if key in _CACHE:
        return _CACHE[key]

    nc = bacc.Bacc("TRN2", target_bir_lowering=False, debug=False, num_devices=C)

    idx_d = nc.dram_tensor("idx", [P, LB * J], I32, kind="ExternalInput")
    w1t_d = nc.dram_tensor("w1t", [V + 1, D], BF16, kind="ExternalInput")
    w2t_d = nc.dram_tensor("w2t", [D, V], BF16, kind="ExternalInput")
    out_d = nc.dram_tensor("out", [LB * P, V], BF16, kind="ExternalOutput")
    if not zero_bias:
        b1_d = nc.dram_tensor("b1", [2, P, 1], F32, kind="ExternalInput")
        b2_d = nc.dram_tensor("b2", [1, V], F32, kind="ExternalInput")

    # output chunks of 8192 cols; vtiles of 1024 within a chunk (tail 848)
    chunks = [(k, min(CHW, V - k)) for k in range(0, V, CHW)]

    with tile.TileContext(nc) as tc, ExitStack() as ctx:
        const = ctx.enter_context(tc.tile_pool(name="const", bufs=1))
        w2pool = ctx.enter_context(tc.tile_pool(name="w2", bufs=22))
        gpool = ctx.enter_context(tc.tile_pool(name="g8", bufs=2))
        t4pool = ctx.enter_context(tc.tile_pool(name="t4", bufs=2))
        t2pool = ctx.enter_context(tc.tile_pool(name="t2", bufs=2))
        hpool = ctx.enter_context(tc.tile_pool(name="hraw", bufs=2))
        opool = ctx.enter_context(tc.tile_pool(name="out", bufs=4))
        b2pool = ctx.enter_context(tc.tile_pool(name="b2s", bufs=6))
        ps_s = ctx.enter_context(tc.tile_pool(name="ps_s", bufs=2, space="PSUM"))
        ps_b = ctx.enter_context(tc.tile_pool(name="ps_b", bufs=3, space="PSUM"))

        # ---- resident tensors -------------------------------------------
        idx_sb = const.tile([P, LB * J], I32, tag="idx")
        nc.sync.dma_start(idx_sb[:], idx_d[:])
        ident = const.tile([P, P], BF16, tag="ident")
        make_identity(nc, ident[:])
        if not zero_bias:
            b1t = [const.tile([P, 1], F32, tag=f"b1{h}", name=f"b1{h}")
                   for h in (0, 1)]
            for h in (0, 1):
                nc.sync.dma_start(b1t[h][:], b1_d[h])

        # ---- layer 1: own two blocks ------------------------------------
        def layer1(lb):
            # pairwise tree: each add only waits on two gathers, so the
            # reduction races the (serial, ~1.1us each) gather issue stream
            g8 = gpool.tile([P, J, D], BF16, tag=f"g8{lb}", name="g8")
            for j in range(J):
                nc.gpsimd.indirect_dma_start(
                    out=g8[:, j, :],
                    out_offset=None,
                    in_=w1t_d[:],
                    in_offset=bass.IndirectOffsetOnAxis(
                        ap=idx_sb[:, lb * J + j:lb * J + j + 1], axis=0),
                )
            t4 = t4pool.tile([P, 4, D], BF16, tag=f"t4{lb}", name="t4")
            for q in range(4):
                nc.vector.tensor_add(t4[:, q, :], g8[:, 2 * q, :],
                                     g8[:, 2 * q + 1, :])
            t2 = t2pool.tile([P, 2, D], BF16, tag=f"t2{lb}", name="t2")
            for q in range(2):
                nc.vector.tensor_add(t2[:, q, :], t4[:, 2 * q, :],
                                     t4[:, 2 * q + 1, :])
            h_raw = hpool.tile([P, D], BF16, tag=f"hraw{lb}", name="h_raw")
            nc.vector.tensor_add(h_raw[:], t2[:, 0, :], t2[:, 1, :])

            hts = []
            for h in (0, 1):
                pt = ps_s.tile([P, P], BF16, tag="ps", name="pt")
                nc.tensor.transpose(pt[:], h_raw[:, h * P:(h + 1) * P],
                                    ident[:])
                ht = const.tile([P, P], BF16, tag=f"ht{lb}{h}",
                                name=f"ht{lb}{h}")
                if zero_bias:
                    nc.vector.tensor_scalar_max(ht[:], pt[:], 0.0)
                else:
                    nc.scalar.activation(ht[:], pt[:],
                                         mybir.ActivationFunctionType.Relu,
                                         bias=b1t[h][:], scale=1.0)
                hts.append(ht)
            return hts

        ht = [layer1(lb) for lb in range(LB)]

        # ---- layer 2: stream W2T, both blocks per vtile ------------------
        def fetch_w2(v0, vw):
            pair = []
            for h in (0, 1):
                w2 = w2pool.tile([P, VT], BF16, tag="w2", name="w2")
                nc.sync.dma_start(w2[:, :vw], w2t_d[h * P:(h + 1) * P,
                                                    v0:v0 + vw])
                pair.append(w2)
            if not zero_bias:
                b2s = b2pool.tile([P, VT], F32, tag="b2s", name="b2s")
                nc.sync.dma_start(b2s[:, :vw],
                                  b2_d[:, v0:v0 + vw].to_broadcast([P, vw]))
                pair.append(b2s)
            return pair

        vtiles = []
        for k0, kw in chunks:
            for v0 in range(k0, k0 + kw, VT):
                vtiles.append((v0, min(VT, k0 + kw - v0)))

        PREF = 10
        OFF = 4            # block B lags A by 4 vtiles: A's matmuls start
                           # as soon as ht_A exists, ~13us before ht_B
        vt_n = len(vtiles)
        w2f = {i: fetch_w2(*vtiles[i]) for i in range(min(PREF, vt_n))}
        obs = {}

        def do_block(lb, i):
            v0, vw = vtiles[i]
            k0 = (v0 // CHW) * CHW
            kw = min(CHW, V - k0)
            if (lb, k0) not in obs:
                obs[(lb, k0)] = opool.tile([P, CHW], BF16, tag=f"ob{lb}",
                                           name=f"ob{lb}")
            ob = obs[(lb, k0)]
            w2pair = w2f[i]
            po = ps_b.tile([P, VT], F32, tag="po", name="po")
            for sub in range(0, vw, 512):
                sw = min(512, vw - sub)
                for h in (0, 1):
                    nc.tensor.matmul(
                        po[:, sub:sub + sw],
                        lhsT=ht[lb][h][:],
                        rhs=w2pair[h][:, sub:sub + sw],
                        start=(h == 0),
                        stop=(h == 1))
            c0 = v0 - k0
            if not zero_bias:
                nc.vector.tensor_add(ob[:, c0:c0 + vw], po[:, :vw],
                                     w2pair[2][:, :vw])
            elif (lb + i) % 2 == 0:
                nc.vector.tensor_scalar_add(ob[:, c0:c0 + vw], po[:, :vw], 0.0)
            else:
                nc.scalar.copy(ob[:, c0:c0 + vw], po[:, :vw])
            if lb == 1:
                w2f.pop(i)
            if v0 + vw == k0 + kw:        # chunk complete for this block
                eng = nc.sync if lb == 0 else nc.scalar
                eng.dma_start(out_d[lb * P:(lb + 1) * P, k0:k0 + kw],
                              ob[:, :kw])
                del obs[(lb, k0)]

        for st in range(vt_n + OFF):
            if st < vt_n:
                if st + PREF < vt_n:
                    w2f[st + PREF] = fetch_w2(*vtiles[st + PREF])
                do_block(0, st)
            if st >= OFF:
                do_block(1, st - OFF)

    nc.compile()
    _CACHE[key] = nc
    return nc


def _host_prep(inputs, W1, b1, W2, b2, zero_bias):
    x = np.asarray(inputs)
    assert x.shape == (N, J) and x.dtype == np.int32

    # duplicate mask: scatter-SET semantics -> only first occurrence counts;
    # duplicates are redirected to the all-zero row V of the augmented W1T.
    dup = np.zeros((N, J), dtype=bool)
    for j in range(1, J):
        dup[:, j] = (x[:, :j] == x[:, j:j + 1]).any(axis=1)
    xd = np.where(dup, V, x).astype(np.int32)

    w1 = np.asarray(W1, dtype=np.float32)
    w1t = np.concatenate([w1.T / J, np.zeros((1, D), np.float32)], axis=0)
    w1t = np.ascontiguousarray(w1t).astype(ml_dtypes.bfloat16)   # [V+1, D]

    w2t = np.ascontiguousarray(
        np.asarray(W2, dtype=np.float32).T).astype(ml_dtypes.bfloat16)

    in_maps = []
    for c in range(C):
        # idx2[p, lb*J + j] = xd[c*256 + lb*128 + p, j]
        xc = xd[c * LB * P:(c + 1) * LB * P]
        idx2 = np.ascontiguousarray(
            xc.reshape(LB, P, J).transpose(1, 0, 2).reshape(P, LB * J))
        m = {
            "idx": idx2,
            "w1t": w1t,
            "w2t": w2t,
        }
        if not zero_bias:
            m["b1"] = np.ascontiguousarray(
                np.asarray(b1, dtype=np.float32).reshape(2, P, 1))
            m["b2"] = np.ascontiguousarray(
                np.asarray(b2, dtype=np.float32).reshape(1, V))
        in_maps.append(m)
    return in_maps


def run(inputs, W1, b1, W2, b2, trace=False):
    from concourse.bass_utils import run_bass_kernel_spmd

    zero_bias = not (np.any(np.asarray(b1)) or np.any(np.asarray(b2)))
    nc = _build(zero_bias)
    in_maps = _host_prep(inputs, W1, b1, W2, b2, zero_bias)
    res = run_bass_kernel_spmd(nc, in_maps, core_ids=list(range(C)), trace=trace)
    out = np.concatenate([res.results[c]["out"] for c in range(C)], axis=0)
    return out.astype(np.float32), res


def kernel(inputs, W1, b1, W2, b2):
    out, _ = run(inputs, W1, b1, W2, b2, trace=False)
    return out
